# revision 1
# baseline (speedup 1.0000x reference)
"""Bipartite GNN message-passing kernel for 8 TRN2 NeuronCores.

Strategy:
  - Host: sort edges by destination node (cons for pass 1, vars for pass 2),
    shard edges across cores by 128-node blocks, pad each block to a uniform
    tile count (SPMD requires one program shape).
  - Device (per core):
      * build node tables (c = relu(cons @ Wc + bc), v = relu(vars @ Wv + bv))
        feature-major via PE, transpose to row-major bf16 via xbar DMA-transpose
      * pass 1: per 512-edge supertile: indirect-gather c/v rows, PE-transpose
        to feature-major, 2-layer join MLP (block-diagonal weights, 4 groups),
        transpose back, one-hot (is_equal) segment-sum matmul accumulated in
        PSUM per 128-node block -> out_c (feature-major), rep-layer -> oc2
      * AllGather oc2 row-major slices across 8 cores
      * pass 2: same structure keyed by vars -> out_v, rep-layer, output MLP
  - Host: concatenate per-core output slices.
"""

import sys

sys.path.insert(0, "/opt/trn_rl_repo")

import numpy as np
import ml_dtypes

import concourse.bass as bass
import concourse.tile as tile
from concourse import bacc, mybir
from concourse import bass_utils

BF16 = ml_dtypes.bfloat16
P = 128
D = 32
ST = 4  # tiles per supertile

F32 = mybir.dt.float32
BF = mybir.dt.bfloat16
I32 = mybir.dt.int32
Relu = mybir.ActivationFunctionType.Relu
Copy = mybir.ActivationFunctionType.Copy


# ----------------------------------------------------------------------------
# host-side preprocessing
# ----------------------------------------------------------------------------

def _prep_pass(sort_ids, cores, bpc):
    """Sort edges by sort_ids, group into 128-node blocks, pad each block to a
    uniform multiple-of-ST tile count.  Returns (order, pos, nblk, T, Lb)."""
    E = sort_ids.shape[0]
    nblk = cores * bpc
    order = np.argsort(sort_ids, kind="stable")
    sk = sort_ids[order].astype(np.int64)
    blk = sk // P
    cnt = np.bincount(blk, minlength=nblk).astype(np.int64)
    T = max(ST, int(np.ceil(cnt.max() / P)))
    T = (T + ST - 1) // ST * ST
    Lb = T * P
    starts = np.zeros(nblk + 1, np.int64)
    np.cumsum(cnt, out=starts[1:])
    rank = np.arange(E, dtype=np.int64) - starts[blk]
    pos = blk * Lb + rank  # slot = blk*Lb + t*P + p with t=rank//P, p=rank%P
    return order, pos, sk, blk, nblk, T, Lb


def _pack_rid(sk, blk, pos, nblk, T, cores, bpc):
    rid = np.full(nblk * T * P, 999.0, np.float32)
    rid[pos] = (sk - blk * P).astype(np.float32)
    return np.ascontiguousarray(
        rid.astype(BF16).reshape(cores, bpc, T, P).transpose(0, 1, 3, 2))


def _pack_feats(feat_tab, node_ids, order, pos, nblk, T, cores, bpc):
    """[cores, bpc, T//ST, ST*F, P] bf16 stacked per supertile group."""
    F = feat_tab.shape[1]
    arr = np.zeros((nblk * T * P, F), np.float32)
    arr[pos] = feat_tab[node_ids[order]]
    arr = arr.astype(BF16).reshape(cores, bpc, T // ST, ST, P, F)
    return np.ascontiguousarray(arr.transpose(0, 1, 2, 3, 5, 4).reshape(
        cores, bpc, T // ST, ST * F, P))


def _pack_idx(node_ids, order, pos, nblk, T, cores, bpc):
    """[cores, bpc, T, P] int32, tile-major rows."""
    arr = np.zeros(nblk * T * P, np.int32)
    arr[pos] = node_ids[order].astype(np.int32)
    return np.ascontiguousarray(arr.reshape(cores, bpc, T, P))


def preprocess(edge_indices, cons_features, vars_features, cores, bpc):
    ec = np.asarray(edge_indices[0]).astype(np.int64)
    ev = np.asarray(edge_indices[1]).astype(np.int64)

    # pass 1: sorted by cons
    o1, p1, sk1, bk1, nblk, T1, _ = _prep_pass(ec, cores, bpc)
    cf1 = _pack_feats(cons_features, ec, o1, p1, nblk, T1, cores, bpc)
    vf1 = _pack_feats(vars_features, ev, o1, p1, nblk, T1, cores, bpc)
    rid1 = _pack_rid(sk1, bk1, p1, nblk, T1, cores, bpc)

    # pass 2: sorted by vars
    o2, p2, sk2, bk2, nblk, T2, _ = _prep_pass(ev, cores, bpc)
    vf2 = _pack_feats(vars_features, ev, o2, p2, nblk, T2, cores, bpc)
    ia2 = _pack_idx(ec, o2, p2, nblk, T2, cores, bpc)
    rid2 = _pack_rid(sk2, bk2, p2, nblk, T2, cores, bpc)

    return (cf1, vf1, rid1, T1), (vf2, ia2, rid2, T2)


# ----------------------------------------------------------------------------
# device program
# ----------------------------------------------------------------------------

def build_program(cores, bpc, T1, T2, CF, VF, debug=False):
    npad = cores * bpc * P
    ns = bpc * P
    nc = bacc.Bacc(None, num_devices=cores)

    def inp(name, shape, dt):
        return nc.dram_tensor(name, shape, dt, kind="ExternalInput")

    t = {}
    t["cons_slice"] = inp("cons_slice", [ns, CF], F32)
    t["vars_slice"] = inp("vars_slice", [ns, VF], F32)
    for nm, shp in [
        ("Wc", [CF, D]), ("Wv", [VF, D]), ("Wj1", [2 * D, D]), ("Wj2", [D, D]),
        ("Wcr", [2 * D, D]), ("Wvr", [2 * D, D]), ("Wo1", [D, D]),
        ("Wo2", [D, D]), ("Wo3", [D, 1]),
    ]:
        t[nm] = inp(nm, shp, F32)
    for nm in ["bc", "bv", "bj1", "bj2", "bcr", "bvr", "bo1", "bo2"]:
        t[nm] = inp(nm, [D, 1], F32)
    t["bo3"] = inp("bo3", [1, 1], F32)
    t["cf1"] = inp("cf1", [bpc, T1 // ST, ST * CF, P], BF)
    t["vf1"] = inp("vf1", [bpc, T1 // ST, ST * VF, P], BF)
    t["rid1"] = inp("rid1", [bpc, P, T1], BF)
    t["vf2"] = inp("vf2", [bpc, T2 // ST, ST * VF, P], BF)
    t["ia2"] = inp("ia2", [bpc, T2, P], I32)
    t["rid2"] = inp("rid2", [bpc, P, T2], BF)
    t["ident"] = inp("ident", [P, P], BF)
    t["iota"] = inp("iota", [1, P], BF)

    out_t = nc.dram_tensor("out", [1, ns], F32, kind="ExternalOutput")

    oc2_row_slice = nc.dram_tensor("oc2_row_slice", [ns, D], BF, kind="Internal")
    oc2_row_full = nc.dram_tensor(
        "oc2_row_full", [npad, D], BF, kind="Internal",
        addr_space="Shared" if cores > 4 else "Local")

    dbg = {}
    if debug:
        for nm, src_t, shp in [("dbg_oc2s", oc2_row_slice, [ns, D]),
                               ("dbg_oc2f", oc2_row_full, [npad, D])]:
            dbg[nm] = (nc.dram_tensor(nm, shp, BF, kind="ExternalOutput"), src_t)
    with tile.TileContext(nc) as tc:
        _emit(tc, t, out_t, oc2_row_slice, oc2_row_full,
              cores, bpc, T1, T2, CF, VF)
        for nm, (dst, src_t) in dbg.items():
            nc.sync.dma_start(out=dst[:], in_=src_t[:])
    nc.compile()
    return nc


def _bcast_row(ap, parts=P):
    """AP reading a [1, N] dram row replicated across `parts` partitions."""
    return bass.AP(tensor=ap.tensor, offset=ap.offset, ap=[[0, parts]] + ap.ap[1:])


def _emit(tc, t, out_t, oc2_row_slice, oc2_row_full, cores, bpc, T1, T2, CF, VF):
    nc = tc.nc
    npad = cores * bpc * P
    ns = bpc * P
    from contextlib import ExitStack
    es = ExitStack()
    singles = es.enter_context(tc.tile_pool(name="singles", bufs=1))

    # ---- constants in SBUF ----
    ident = singles.tile([P, P], BF)
    nc.sync.dma_start(out=ident[:], in_=t["ident"][:])
    iota = singles.tile([P, P], BF)
    nc.sync.dma_start(out=iota[:], in_=_bcast_row(t["iota"][:]))

    def load_w(nm, shape, src):
        w = singles.tile(shape, BF, tag=nm)
        nc.gpsimd.dma_start(out=w[:], in_=src)
        return w

    Wc_sb = load_w("Wc", [CF, D], t["Wc"][:])
    Wv_sb = load_w("Wv", [VF, D], t["Wv"][:])
    WcrT_sb = load_w("WcrT", [D, D], t["Wcr"][0:D, :])
    WcrB_sb = load_w("WcrB", [D, D], t["Wcr"][D:2 * D, :])
    WvrT_sb = load_w("WvrT", [D, D], t["Wvr"][0:D, :])
    WvrB_sb = load_w("WvrB", [D, D], t["Wvr"][D:2 * D, :])
    Wo1_sb = load_w("Wo1", [D, D], t["Wo1"][:])
    Wo2_sb = load_w("Wo2", [D, D], t["Wo2"][:])
    Wo3_sb = load_w("Wo3", [D, 1], t["Wo3"][:])

    # block-diagonal weights
    def blockdiag(nm, src, F_, N_):
        bd = singles.tile([ST * F_, P], BF, tag="bd" + nm)
        nc.vector.memset(bd[:], 0)
        for g in range(ST):
            nc.gpsimd.dma_start(out=bd[g * F_:(g + 1) * F_, g * N_:(g + 1) * N_],
                                in_=src)
        return bd

    bdWc = blockdiag("Wc", t["Wc"][:], CF, D)
    bdWv = blockdiag("Wv", t["Wv"][:], VF, D)
    bdA = blockdiag("WjA", t["Wj1"][0:D, :], D, D)
    bdB = blockdiag("WjB", t["Wj1"][D:2 * D, :], D, D)
    bdW2 = blockdiag("Wj2", t["Wj2"][:], D, D)

    def bias4(nm):
        b = singles.tile([P, 1], F32, tag="b4_" + nm)
        for g in range(4):
            nc.sync.dma_start(out=b[g * D:(g + 1) * D, :], in_=t[nm][:])
        return b

    bj1_4 = bias4("bj1")
    bj2_4 = bias4("bj2")
    bc_4 = bias4("bc")
    bv_4 = bias4("bv")
    bcr_sb = singles.tile([D, 1], F32, tag="bcr")
    nc.sync.dma_start(out=bcr_sb[:], in_=t["bcr"][:])
    bvr_sb = singles.tile([D, 1], F32, tag="bvr")
    nc.sync.dma_start(out=bvr_sb[:], in_=t["bvr"][:])
    bo1_sb = singles.tile([D, 1], F32, tag="bo1")
    nc.sync.dma_start(out=bo1_sb[:], in_=t["bo1"][:])
    bo2_sb = singles.tile([D, 1], F32, tag="bo2")
    nc.sync.dma_start(out=bo2_sb[:], in_=t["bo2"][:])
    bo3_sb = singles.tile([1, 1], F32, tag="bo3")
    nc.sync.dma_start(out=bo3_sb[:], in_=t["bo3"][:])

    # persistent feature-major node slices
    cT_slice = singles.tile([D, ns], BF)
    vT_slice = singles.tile([D, ns], BF)
    oc2T_slice = singles.tile([D, ns], BF)

    # ---- stage A: per-core node tables (feature-major slices only) ----
    with tc.tile_pool(name="build", bufs=3) as bpool, \
         tc.tile_pool(name="bpsum", bufs=3, space="PSUM") as bpsum:

        def build_T(dst, n_rows, feat_dram, F_, W_sb, b4):
            for st0 in range(0, n_rows, 512):
                w = min(512, n_rows - st0)
                ft = bpool.tile([VF, 512], BF, tag="ft")
                nc.gpsimd.dma_start(
                    out=ft[:F_, :w],
                    in_=feat_dram[st0:st0 + w, :].rearrange("n f -> f n"),
                )
                ps = bpsum.tile([D, 512], F32, tag="ps")
                nc.tensor.matmul(out=ps[:, :w], lhsT=W_sb[:F_, :],
                                 rhs=ft[:F_, :w], start=True, stop=True)
                nc.scalar.activation(dst[:, st0:st0 + w], ps[:, :w],
                                     Relu, bias=b4[0:D, :])

        build_T(cT_slice, ns, t["cons_slice"], CF, Wc_sb, bc_4)
        build_T(vT_slice, ns, t["vars_slice"], VF, Wv_sb, bv_4)

    # ---- join pass emitter ----
    def join_pass(T, cfeat_d, vfeat_d, rid_d, FA, gather_tab, idxA_d, rep_cb):
        """FA: A-side raw feature count (pass 1) or None for gather mode."""
        with tc.tile_pool(name="blk", bufs=2) as blkp, \
             tc.tile_pool(name="stt", bufs=3) as stp, \
             tc.tile_pool(name="gth", bufs=10) as gthp, \
             tc.tile_pool(name="wps", bufs=5, space="PSUM") as wps, \
             tc.tile_pool(name="ops", bufs=2, space="PSUM") as ops:
            dma_engines = [nc.sync, nc.scalar]
            for b in range(bpc):
                rid = blkp.tile([P, T], BF, tag="rid")
                nc.sync.dma_start(out=rid[:], in_=rid_d[b])
                out_blk = ops.tile([D, P], F32, tag="ob")
                nst = T // ST
                for s in range(nst):
                    # ---- A side ----
                    if FA is not None:
                        cf = blkp.tile([ST * FA, P], BF, tag="cf")
                        nc.sync.dma_start(out=cf[:], in_=cfeat_d[b, s])
                        psAf = wps.tile([P, P], F32, tag="w")
                        nc.tensor.matmul(out=psAf[:], lhsT=bdWc[:], rhs=cf[:],
                                         start=True, stop=True)
                        xA = stp.tile([P, P], BF, tag="xa")
                        nc.scalar.activation(xA[:], psAf[:], Relu, bias=bc_4[:])
                    else:
                        psA = wps.tile([P, P], BF, tag="w")
                        for g in range(ST):
                            tt = s * ST + g
                            it = gthp.tile([P, 1], I32, tag="ix")
                            dma_engines[tt % 2].dma_start(
                                out=it[:], in_=idxA_d[b, tt, :, None])
                            gO = gthp.tile([P, D], BF, tag="gO")
                            nc.gpsimd.indirect_dma_start(
                                out=gO[:], out_offset=None, in_=gather_tab[:],
                                in_offset=bass.IndirectOffsetOnAxis(
                                    ap=it[:], axis=0))
                            nc.tensor.matmul(
                                out=psA[g * D:(g + 1) * D, :], lhsT=gO[:],
                                rhs=ident[:], is_transpose=True,
                                tile_position=(0, g * D))
                        xA = stp.tile([P, P], BF, tag="xa")
                        nc.scalar.activation(xA[:], psA[:], Copy)
                    # ---- B side (always raw vars features) ----
                    vf = blkp.tile([ST * VF, P], BF, tag="vf")
                    nc.sync.dma_start(out=vf[:], in_=vfeat_d[b, s])
                    psB = wps.tile([P, P], F32, tag="w")
                    nc.tensor.matmul(out=psB[:], lhsT=bdWv[:], rhs=vf[:],
                                     start=True, stop=True)
                    xB = stp.tile([P, P], BF, tag="xb")
                    nc.scalar.activation(xB[:], psB[:], Relu, bias=bv_4[:])
                    # ---- join MLP ----
                    ph = wps.tile([P, P], F32, tag="w")
                    nc.tensor.matmul(out=ph[:], lhsT=bdA[:], rhs=xA[:],
                                     start=True, stop=False)
                    nc.tensor.matmul(out=ph[:], lhsT=bdB[:], rhs=xB[:],
                                     start=False, stop=True)
                    h1 = stp.tile([P, P], BF, tag="h1")
                    nc.scalar.activation(h1[:], ph[:], Relu, bias=bj1_4[:])
                    pj = wps.tile([P, P], F32, tag="w")
                    nc.tensor.matmul(out=pj[:], lhsT=bdW2[:], rhs=h1[:],
                                     start=True, stop=True)
                    jT = stp.tile([P, P], BF, tag="jt")
                    nc.scalar.activation(jT[:], pj[:], Relu, bias=bj2_4[:])
                    pjt = wps.tile([P, P], BF, tag="w")
                    nc.tensor.transpose(pjt[:], jT[:], ident[:])
                    j4 = stp.tile([P, P], BF, tag="j4")
                    nc.scalar.activation(j4[:], pjt[:], Copy)
                    for g in range(ST):
                        S = stp.tile([P, P], BF, tag=f"s{g}")
                        k = s * ST + g
                        nc.vector.tensor_tensor(
                            out=S[:], in0=rid[:, k:k + 1].to_broadcast([P, P]),
                            in1=iota[:], op=mybir.AluOpType.is_equal,
                        )
                        nc.tensor.matmul(
                            out=out_blk[:], lhsT=j4[:, g * D:(g + 1) * D],
                            rhs=S[:],
                            start=(s == 0 and g == 0),
                            stop=(s == nst - 1 and g == ST - 1),
                            skip_group_check=True,
                        )
                rep_cb(b, out_blk, stp, wps)

    # ---- pass 1 ----
    def rep1(b, out_blk, stp, wps):
        oc_in = stp.tile([D, P], BF, tag="rin")
        nc.scalar.activation(oc_in[:], out_blk[:], Copy)
        ps = wps.tile([D, P], F32, tag="w")
        nc.tensor.matmul(out=ps[:], lhsT=WcrT_sb[:], rhs=oc_in[:],
                         start=True, stop=False)
        nc.tensor.matmul(out=ps[:], lhsT=WcrB_sb[:],
                         rhs=cT_slice[:, b * P:(b + 1) * P],
                         start=False, stop=True)
        nc.scalar.activation(oc2T_slice[:, b * P:(b + 1) * P], ps[:],
                             Relu, bias=bcr_sb[:])

    join_pass(T1, t["cf1"], t["vf1"], t["rid1"], CF, None, None, rep1)

    # oc2 row-major slice + exchange
    with tc.tile_pool(name="xch", bufs=1) as xp:
        stg = xp.tile([P, bpc, D], BF)
        nc.sync.dma_start(out=stg[:], in_=oc2T_slice[:], transpose=True)
        nc.sync.dma_start(
            out=oc2_row_slice[:].rearrange("(j p) d -> p j d", p=P),
            in_=stg[:],
        )
    if cores > 1:
        nc.gpsimd.collective_compute(
            "AllGather", mybir.AluOpType.bypass,
            replica_groups=[list(range(cores))],
            ins=[oc2_row_slice[:]], outs=[oc2_row_full[:]],
        )
    else:
        nc.sync.dma_start(out=oc2_row_full[:], in_=oc2_row_slice[:])

    # ---- pass 2 ----
    def rep2(b, out_blk, stp, wps):
        ov_in = stp.tile([D, P], BF, tag="rin")
        nc.scalar.activation(ov_in[:], out_blk[:], Copy)
        ps = wps.tile([D, P], F32, tag="w")
        nc.tensor.matmul(out=ps[:], lhsT=WvrT_sb[:], rhs=ov_in[:],
                         start=True, stop=False)
        nc.tensor.matmul(out=ps[:], lhsT=WvrB_sb[:],
                         rhs=vT_slice[:, b * P:(b + 1) * P],
                         start=False, stop=True)
        ov2 = stp.tile([D, P], BF, tag="ov2")
        nc.scalar.activation(ov2[:], ps[:], Relu, bias=bvr_sb[:])
        p1 = wps.tile([D, P], F32, tag="w")
        nc.tensor.matmul(out=p1[:], lhsT=Wo1_sb[:], rhs=ov2[:],
                         start=True, stop=True)
        h1o = stp.tile([D, P], BF, tag="h1o")
        nc.scalar.activation(h1o[:], p1[:], Relu, bias=bo1_sb[:])
        p2 = wps.tile([D, P], F32, tag="w")
        nc.tensor.matmul(out=p2[:], lhsT=Wo2_sb[:], rhs=h1o[:],
                         start=True, stop=True)
        h2o = stp.tile([D, P], BF, tag="h2o")
        nc.scalar.activation(h2o[:], p2[:], Relu, bias=bo2_sb[:])
        p3 = wps.tile([1, P], F32, tag="w")
        nc.tensor.matmul(out=p3[:], lhsT=Wo3_sb[:], rhs=h2o[:],
                         start=True, stop=True)
        outB = stp.tile([1, P], F32, tag="outB")
        nc.vector.tensor_scalar_add(outB[:], p3[:], bo3_sb[:])
        nc.sync.dma_start(out=out_t[:, b * P:(b + 1) * P], in_=outB[:])

    join_pass(T2, None, t["vf2"], t["rid2"], None, oc2_row_full, t["ia2"], rep2)
    es.close()


def make_in_maps(inputs, cores, bpc, CF, VF, n_nodes):
    npad = cores * bpc * P
    ns = bpc * P

    cons = np.zeros((npad, CF), np.float32)
    cons[:n_nodes] = np.asarray(inputs["cons_features"], np.float32)
    varsf = np.zeros((npad, VF), np.float32)
    varsf[:n_nodes] = np.asarray(inputs["vars_features"], np.float32)

    (cf1, vf1, rid1, T1), (vf2, ia2, rid2, T2) = preprocess(
        np.asarray(inputs["edge_indices"]), cons, varsf, cores, bpc)

    com = {}
    for nm in ["Wc", "Wv", "Wj1", "Wj2", "Wcr", "Wvr", "Wo1", "Wo2", "Wo3"]:
        com[nm] = np.ascontiguousarray(np.asarray(inputs[nm], np.float32))
    for nm in ["bc", "bv", "bj1", "bj2", "bcr", "bvr", "bo1", "bo2"]:
        com[nm] = np.asarray(inputs[nm], np.float32).reshape(D, 1)
    com["bo3"] = np.asarray(inputs["bo3"], np.float32).reshape(1, 1)
    com["ident"] = np.eye(P, dtype=BF16)
    com["iota"] = np.arange(P, dtype=BF16).reshape(1, P)

    in_maps = []
    for c in range(cores):
        m = dict(com)
        m["cons_slice"] = np.ascontiguousarray(cons[c * ns:(c + 1) * ns])
        m["vars_slice"] = np.ascontiguousarray(varsf[c * ns:(c + 1) * ns])
        m["cf1"], m["vf1"], m["rid1"] = cf1[c], vf1[c], rid1[c]
        m["vf2"], m["ia2"], m["rid2"] = vf2[c], ia2[c], rid2[c]
        in_maps.append(m)
    return in_maps, T1, T2


def _pjrt_run(nc, in_maps, cores, iters=1):
    """Compile once via PJRT, execute `iters` times, return (out_list, times)."""
    import time
    import jax
    from jax.experimental.shard_map import shard_map
    from jax.sharding import Mesh, PartitionSpec
    from concourse import bass2jax
    from concourse.bass2jax import (
        install_neuronx_cc_hook, partition_id_tensor, _bass_exec_p)

    install_neuronx_cc_hook()
    assert nc.dbg_addr is None or not nc.dbg_callbacks
    if nc.dbg_addr is not None:
        in_maps = [
            {**m, nc.dbg_addr.name: np.zeros((1, 2), np.uint32)} for m in in_maps
        ]
    partition_name = nc.partition_id_tensor.name if nc.partition_id_tensor else None

    in_names, out_names, out_avals, zero_outs = [], [], [], []
    for alloc in nc.m.functions[0].allocations:
        if not isinstance(alloc, mybir.MemoryLocationSet):
            continue
        name = alloc.memorylocations[0].name
        if alloc.kind == "ExternalInput":
            if name != partition_name:
                in_names.append(name)
        elif alloc.kind == "ExternalOutput":
            shape = tuple(alloc.tensor_shape)
            dtype = mybir.dt.np(alloc.dtype)
            out_names.append(name)
            out_avals.append(jax.core.ShapedArray(shape, dtype))
            zero_outs.append(np.zeros(shape, dtype))
    n_params = len(in_names)
    n_outs = len(out_avals)
    all_in_names = list(in_names) + list(out_names)
    if partition_name is not None:
        all_in_names.append(partition_name)

    def _body(*args):
        operands = list(args)
        if partition_name is not None:
            operands.append(partition_id_tensor())
        outs = _bass_exec_p.bind(
            *operands,
            out_avals=tuple(out_avals),
            in_names=tuple(all_in_names),
            out_names=tuple(out_names),
            lowering_input_output_aliases=(),
            sim_require_finite=True,
            sim_require_nnan=True,
            nc=nc,
        )
        return tuple(outs)

    devices = jax.devices()[:cores]
    mesh = Mesh(np.asarray(devices), ("core",))
    in_specs = (PartitionSpec("core"),) * (n_params + n_outs)
    out_specs = (PartitionSpec("core"),) * len(out_names)
    sharded = jax.jit(
        shard_map(_body, mesh=mesh, in_specs=in_specs, out_specs=out_specs,
                  check_rep=False),
        donate_argnums=tuple(range(n_params, n_params + n_outs)),
        keep_unused=True,
    )
    concat_in = [
        np.concatenate([np.asarray(in_maps[c][nm]) for c in range(cores)], axis=0)
        for nm in in_names
    ]
    # keep inputs device-resident across iterations
    from jax.sharding import NamedSharding
    shard = NamedSharding(mesh, PartitionSpec("core"))
    dev_in = [jax.device_put(a, shard) for a in concat_in]

    def zeros():
        return [np.zeros((cores * z.shape[0], *z.shape[1:]), z.dtype)
                for z in zero_outs]

    out_arrs = sharded(*dev_in, *zeros())
    jax.block_until_ready(out_arrs)
    times = []
    for _ in range(max(0, iters - 1)):
        t0 = time.perf_counter()
        out_arrs2 = sharded(*dev_in, *zeros())
        jax.block_until_ready(out_arrs2)
        times.append(time.perf_counter() - t0)
    results = [
        {nm: np.asarray(out_arrs[i]).reshape(cores, *out_avals[i].shape)[c]
         for i, nm in enumerate(out_names)}
        for c in range(cores)
    ]
    return results, times


def run(inputs, cores, bpc, n_nodes, iters=1, debug=False):
    CF = np.asarray(inputs["cons_features"]).shape[1]
    VF = np.asarray(inputs["vars_features"]).shape[1]
    in_maps, T1, T2 = make_in_maps(inputs, cores, bpc, CF, VF, n_nodes)
    nc = build_program(cores, bpc, T1, T2, CF, VF, debug=debug)
    results, times = _pjrt_run(nc, in_maps, cores, iters=iters)
    out = np.concatenate([results[c]["out"].reshape(-1) for c in range(cores)])
    out = out[:n_nodes].reshape(n_nodes, 1).astype(np.float32)
    if debug:
        return out, times, results
    return out, times


def kernel(**inputs) -> np.ndarray:
    out, _ = run(inputs, cores=8, bpc=98, n_nodes=100_000)
    return out

